# revision 9
# baseline (speedup 1.0000x reference)
"""Trainium2 Bass kernel for the quantized ResNet Bottleneck block.

Data parallel over batch: 64 images -> 8 cores x 8 images (4 pairs/core).
All convs are integer-valued bf16 matmuls accumulated in fp32 PSUM (exact:
activation codes live in [1,255] after a +128 shift; the shift's constants
and all quant scales are folded into biases on host).

Design (each point validated against a perfetto/NTFF trace):
  * x fake-quant on host; bf16 codes upload directly (halves input DMA,
    deletes two on-chip elementwise passes). fp16 output download.
  * Layer-blocked schedule: conv1(p0,p1) -> conv2(p0,p1) -> conv1(p2,p3)
    -> conv2(p2,p3) -> conv3+shortcut(all); the conv2 half-phases cover
    the window where pair-2/3 input DMAs are still in flight. Weight
    blobs split (w1|w2|w3+ws) so each arrives before its layer.
  * conv3 and the stride-2 shortcut share one PSUM accumulation
    (shortcut weights pre-scaled by css/c3s on host).
  * Single 8-buffer rotating PSUM pool; fused single-pass DVE epilogues
    (QEPI magic-constant round/clip, FEPI relu6+fp16); border-only pad
    memsets on GPSIMD.
  * PE p-state warmup matmuls bridge the ~6us DMA cold-start; small
    leading DMAs absorb per-queue first-descriptor latency.
  * Duplicate LDWEIGHTS removed post-scheduling (KERNEL_NO_DEDUP=1
    disables; measured ~1.7us benefit).
"""

import os
import sys
from contextlib import ExitStack

import numpy as np

sys.path.insert(0, "/opt/trn_rl_repo")

import ml_dtypes  # noqa: E402

import concourse.bacc as bacc  # noqa: E402
import concourse.dve_ops as dve_ops  # noqa: E402
import concourse.tile as tile  # noqa: E402
from concourse import mybir  # noqa: E402
from concourse.bass_utils import run_bass_kernel_spmd  # noqa: E402
from concourse.dve_spec import (  # noqa: E402
    C0 as DC0,
    C1 as DC1,
    C2 as DC2,
    One as DOne,
    Spec,
    Src0 as DSrc0,
    Src1 as DSrc1,
    _has_src1,
    lower as dve_lower,
    maxx,
    minn,
    relu as drelu,
)
from concourse.dve_uop import DveOpSpec  # noqa: E402
from concourse.dve_table_gen import dve_ver_for  # noqa: E402, F401
from concourse.dve_ops import DveOp  # noqa: E402

F32 = mybir.dt.float32
F16 = mybir.dt.float16
BF16 = mybir.dt.bfloat16
ALU = mybir.AluOpType
AFT = mybir.ActivationFunctionType
BF16NP = ml_dtypes.bfloat16

C_MAGIC = float(np.float32(12582912.0))  # 1.5 * 2**23

N_CORES = 8
B_LOC = 8
PAIRS = 4

DEDUP_LD = os.environ.get("KERNEL_NO_DEDUP", "") != "1"


def _register_dve_op(name, spec, subdim=False):
    for o in dve_ops.OPS:
        if o.name == name:
            return o
    row = dve_ops._CUSTOM_DVE_ROW_BASE + len(dve_ops.OPS)
    assert row < 0x20
    shas = {}
    for ver in ("v3", "v4"):
        tmp = DveOpSpec(
            name=name, opcode=row, uops=dve_lower(spec, ver=ver),
            rd1_en=_has_src1(spec),
        )
        shas[ver] = tmp.sha(ver)
    op = DveOp(name, spec, subdim=subdim, uops_sha=shas)
    dve_ops.OPS.append(op)
    dve_ops._SUB_OPCODE_FOR_NAME[name] = row
    dve_ops.CUSTOM_DVE_SPECS[name] = spec
    return op


def _b(in0, in1):
    if isinstance(in1, np.ndarray) and in1.size == in0.size:
        return in1.reshape(in0.shape)
    return in1


# quantize epilogue: v = in0*alpha + beta ; out = clip(round(v),0,127) + 128
QEPI = _register_dve_op(
    "BNECK_QEPI_ANT",
    Spec(
        body=(minn(maxx((DSrc0 * DC0 + DSrc1) + DC1, DC1), DC1 + DC2) - DC1)
        + (DC2 + DOne),
        reference=lambda in0, in1, s0, s1, imm2: np.minimum(
            np.maximum(np.round(in0 * s0 + _b(in0, in1)), 0.0), imm2
        )
        + (imm2 + 1.0),
    ),
)


# final epilogue: out = min(relu(in0*gamma + delta), 6)
FEPI = _register_dve_op(
    "BNECK_FEPI_ANT",
    Spec(
        body=minn(drelu(DSrc0 * DC0 + DSrc1), DC1),
        reference=lambda in0, in1, s0, s1, imm2: np.minimum(
            np.maximum(in0 * s0 + _b(in0, in1), 0.0), s1
        ),
    ),
)


def dedup_ldweights(nc):
    """Remove InstLdweights that reload the stationary already in the PE.

    Safe because: weights tiles are written once (their DMA dep is carried
    by the group's first ld), matmuls don't reference lds by name, and PE
    queue instructions execute in order.
    """
    removed = 0
    for f in nc.m.functions:
        for b in f.blocks:
            insts = b.instructions
            keep = []
            last_key = None
            for i in insts:
                t = type(i).__name__
                if t == "InstLdweights":
                    key = str(i.ins[0])
                    if key == last_key and not i.has_wait():
                        removed += 1
                        continue
                    last_key = key
                elif t == "InstMatmult":
                    pass
                elif i.engine == mybir.EngineType.PE:
                    last_key = None
                keep.append(i)
            insts[:] = keep
    return removed


def _build_nc(pairs=PAIRS):
    nc = bacc.Bacc("TRN2", target_bir_lowering=False, debug=False)

    # x codes (+128) packed [pair][k-chunk][q][i*784] bf16
    x_d = nc.dram_tensor("xc", [pairs, 4, 128, 1568], BF16, kind="ExternalInput")
    # w1 lhsT slices [(k*2+m)*128 .. +128)
    wa_d = nc.dram_tensor("wa", [128, 1024], BF16, kind="ExternalInput")
    # w2 slices [(tap*2+k)*2+m]
    wb2_d = nc.dram_tensor("wb2", [128, 4608], BF16, kind="ExternalInput")
    # w3 [(k*8+m8)*128], ws [2048 + (k*8+m8)*128]
    wb3_d = nc.dram_tensor("wb3", [128, 6144], BF16, kind="ExternalInput")
    # beta1 [0:2], beta2 [2:4], delta [4:12]
    ms_d = nc.dram_tensor("msc", [128, 12], F32, kind="ExternalInput")
    # fp16 out, per pair [q][m8][i*196], two half-DMAs
    y_d = nc.dram_tensor("y", [pairs, 128, 3136], F16, kind="ExternalOutput")

    a1, a2, g3 = _SCALES

    with tile.TileContext(nc) as tc, ExitStack() as ctx:
        wp = ctx.enter_context(tc.tile_pool(name="w", bufs=1))
        xqp = ctx.enter_context(tc.tile_pool(name="xq", bufs=1))
        p2p = ctx.enter_context(tc.tile_pool(name="p2", bufs=1))
        t3p = ctx.enter_context(tc.tile_pool(name="t3", bufs=1))
        rp = ctx.enter_context(tc.tile_pool(name="r", bufs=4))
        yop = ctx.enter_context(tc.tile_pool(name="yo", bufs=1))
        pp = ctx.enter_context(tc.tile_pool(name="pp", bufs=8, space="PSUM"))

        # ---- one-time loads ----
        xq = []
        for p in range(pairs):
            t = xqp.tile([128, 6272], BF16, name=f"xq{p}")
            xq.append(t)
        wa = wp.tile([128, 1024], BF16, name="wa")
        # first conv1 inputs issued from engine queues that clear the
        # framework preamble earliest
        # tiny first transfer per queue absorbs the DMA first-descriptor
        # latency so the real chunks stream at full rate
        msc = wp.tile([128, 12], F32, name="msc")
        nc.sync.dma_start(msc[:], ms_d[:])
        nc.scalar.dma_start(wa[:, 0:16], wa_d[:, 0:16])
        nc.sync.dma_start(wa[:, 16:1024], wa_d[:, 16:1024])
        nc.scalar.dma_start(xq[0][:, 0:1568], x_d[0, 0])
        nc.sync.dma_start(xq[0][:, 1568:3136], x_d[0, 1])
        nc.scalar.dma_start(xq[0][:, 3136:4704], x_d[0, 2])
        nc.sync.dma_start(xq[0][:, 4704:6272], x_d[0, 3])

        # PE p-state warmup on constant data during the DMA head
        wu = wp.tile([128, 392], BF16, name="wu")
        nc.gpsimd.memset(wu[:], 1.0)
        wups = pp.tile([128, 392], F32, name="ps")
        for w in range(18):
            nc.tensor.matmul(
                wups[:], wu[:, 0:128], wu[:], start=(w == 0), stop=False,
                skip_group_check=True,
            )
        # fine-grained tail so the real work starts within ~65ns of data
        for w in range(24):
            nc.tensor.matmul(
                wups[:, 0:64], wu[:, 0:128], wu[:, 0:64],
                start=False, stop=(w == 23), skip_group_check=True,
            )

        b1f, b2f = [], []
        for m in range(2):
            t = wp.tile([128, 392], F32, name=f"b1f{m}")
            nc.vector.tensor_copy(t[:], msc[:, m : m + 1].to_broadcast((128, 392)))
            b1f.append(t)
        for m in range(2):
            t = wp.tile([128, 392], F32, name=f"b2f{m}")
            nc.vector.tensor_copy(
                t[:], msc[:, 2 + m : 3 + m].to_broadcast((128, 392))
            )
            b2f.append(t)
        d8f = []
        for m8 in range(8):
            t = wp.tile([128, 392], F32, name=f"d8f{m8}")
            nc.vector.tensor_copy(
                t[:], msc[:, 4 + m8 : 5 + m8].to_broadcast((128, 392))
            )
            d8f.append(t)

        wb2 = wp.tile([128, 4608], BF16, name="wb2")
        wb3 = wp.tile([128, 6144], BF16, name="wb3")

        # p2 padded tiles (conv2 input): [q][i(2), 29, 32] per (pair, kc)
        p2 = {}
        for p in range(pairs):
            for kc in range(2):
                t = p2p.tile([128, 1856], BF16, name=f"p2_{p}_{kc}")
                pv = t.rearrange("q (i r c) -> q i r c", i=2, r=29, c=32)
                nc.gpsimd.memset(pv[:, :, 0:1, 1:30], 128.0)
                nc.gpsimd.memset(pv[:, :, 1:29, 1:2], 128.0)
                p2[p, kc] = t

        # ---- conv1: per pair, i-outer, k-outer, (m,hf) groups ----
        def emit_conv1(p):
            xv = xq[p].rearrange("q (k i h) -> q k i h", k=4, i=2)
            for i in (0, 1):
                ps = {}
                for m in (0, 1):
                    for hf in (0, 1):
                        ps[m, hf] = pp.tile([128, 392], F32, name="ps")
                for k in range(4):
                    for m in (0, 1):
                        w_sl = wa[:, (k * 2 + m) * 128 : (k * 2 + m + 1) * 128]
                        for hf in (0, 1):
                            nc.tensor.matmul(
                                ps[m, hf][:],
                                w_sl,
                                xv[:, k, i, hf * 392 : (hf + 1) * 392],
                                start=(k == 0),
                                stop=(k == 3),
                            )
                for m in (0, 1):
                    pv = p2[p, m].rearrange("q (i r c) -> q i r c", i=2, r=29, c=32)
                    for hf in (0, 1):
                        nc.vector._custom_dve(
                            QEPI,
                            out=pv[:, i, 1 + 14 * hf : 15 + 14 * hf, 2:30],
                            in0=ps[m, hf][:].rearrange("q (a b) -> q a b", a=14),
                            in1=b1f[m][:],
                            s0=a1,
                            s1=C_MAGIC,
                            imm2=127.0,
                        )

        for k in range(4):
            nc.sync.dma_start(xq[1][:, k * 1568 : (k + 1) * 1568], x_d[1, k])
        nc.sync.dma_start(wb2[:], wb2_d[:])
        for p in (2, 3):
            for k in range(4):
                nc.sync.dma_start(
                    xq[p][:, k * 1568 : (k + 1) * 1568], x_d[p, k]
                )
        nc.sync.dma_start(wb3[:], wb3_d[:])

        # ---- conv2: 3x3 s2, half the pairs per stationary pass ----
        t3 = {}

        def emit_conv2(plist):
            for m in (0, 1):
                ps2 = {}
                for p in plist:
                    ps2[p] = pp.tile([128, 392], F32, name="ps")
                for k in (0, 1):
                    for tap in range(9):
                        ky, kx = divmod(tap, 3)
                        w_sl = wb2[:, ((tap * 2 + k) * 2 + m) * 128 : ((tap * 2 + k) * 2 + m + 1) * 128]
                        for p in plist:
                            pv = p2[p, k].rearrange(
                                "q (i r c) -> q i r c", i=2, r=29, c=32
                            )
                            nc.tensor.matmul(
                                ps2[p][:],
                                w_sl,
                                pv[:, :, ky : min(ky + 28, 29) : 2, 1 + kx : 29 + kx : 2],
                                start=(k == 0 and tap == 0),
                                stop=(k == 1 and tap == 8),
                            )
                for p in plist:
                    t = t3p.tile([128, 392], BF16, name=f"t3_{p}_{m}")
                    nc.vector._custom_dve(
                        QEPI,
                        out=t[:],
                        in0=ps2[p][:],
                        in1=b2f[m][:],
                        s0=a2,
                        s1=C_MAGIC,
                        imm2=127.0,
                    )
                    t3[p, m] = t

        # schedule: conv2 of pairs (0,1) fills the xq2/xq3 DMA window
        emit_conv1(0)
        emit_conv1(1)
        emit_conv2((0, 1))
        emit_conv1(2)
        emit_conv1(3)
        emit_conv2((2, 3))

        # ---- conv3 (1x1) + stride-2 shortcut into shared PSUM ----
        yo = {}
        for p in range(pairs):
            yo[p] = yop.tile([128, 3136], F16, name=f"yo{p}")
        for m8 in range(8):
            ps3 = {}
            for p in range(pairs):
                ps3[p] = pp.tile([128, 392], F32, name="ps")
            for k in range(4):
                w_sl = wb3[:, 2048 + (k * 8 + m8) * 128 : 2048 + (k * 8 + m8 + 1) * 128]
                for p in range(pairs):
                    xv = xq[p].rearrange(
                        "q (k i r c) -> q k i r c", k=4, i=2, r=28, c=28
                    )
                    nc.tensor.matmul(
                        ps3[p][:],
                        w_sl,
                        xv[:, k, :, 0:28:2, 0:28:2],
                        start=(k == 0),
                        stop=False,
                        skip_group_check=True,
                    )
            for k in (0, 1):
                w_sl = wb3[:, (k * 8 + m8) * 128 : (k * 8 + m8 + 1) * 128]
                for p in range(pairs):
                    nc.tensor.matmul(
                        ps3[p][:],
                        w_sl,
                        t3[p, k][:],
                        start=False,
                        stop=(k == 1),
                        skip_group_check=True,
                    )
            for p in range(pairs):
                nc.vector._custom_dve(
                    FEPI,
                    out=yo[p][:, m8 * 392 : (m8 + 1) * 392],
                    in0=ps3[p][:],
                    in1=d8f[m8][:],
                    s0=g3,
                    s1=6.0,
                )
                if m8 >= 4:
                    nc.sync.dma_start(
                        y_d[p, :, m8 * 392 : (m8 + 1) * 392],
                        yo[p][:, m8 * 392 : (m8 + 1) * 392],
                    )
            if m8 == 3:
                for p in range(pairs):
                    nc.sync.dma_start(y_d[p, :, 0:1568], yo[p][:, 0:1568])

    if DEDUP_LD:
        dedup_ldweights(nc)
    return nc


_SCALES = (1.0, 1.0, 1.0)


def _prep(w1, b1, w2, b2, w3, b3, wsw, bs):
    """Host-side weight quantization + constant folding (all tiny tensors)."""
    f32 = np.float32

    def qw(w):
        s = f32(np.max(np.abs(w)))
        wq = np.round(np.clip(w / s, f32(-1.0), f32(1.0)) * f32(127.0)).astype(
            np.float32
        )
        return wq, s

    def qb(b):
        return np.round(b * f32(127.0)).astype(np.float32)

    w1q, c1s = qw(w1)
    w2q, c2s = qw(w2)
    w3q, c3s = qw(w3)
    wsq, css = qw(wsw)
    B1, B2, B3, Bs = qb(b1), qb(b2), qb(b3), qb(bs)

    a1 = f32(2.0) * c1s / f32(127.0)
    a2 = f32(2.0) * c2s / f32(127.0)
    g3 = c3s / f32(2.0 * 16129.0)
    rho = css / c3s

    # lhsT layouts -> packed blobs
    w1l = w1q[:, :, 0, 0].T.reshape(4, 128, 256)  # [k][cin128][cout256]
    wa = np.zeros((128, 1024), np.float32)
    for k in range(4):
        for m in range(2):
            wa[:, (k * 2 + m) * 128 : (k * 2 + m + 1) * 128] = w1l[k][
                :, m * 128 : (m + 1) * 128
            ]

    w2l = w2q.transpose(2, 3, 1, 0).reshape(9, 2, 128, 256)  # [tap][k][128][256]
    w3l = w3q[:, :, 0, 0].T.reshape(2, 128, 1024)
    ws_sc = (rho * wsq[:, :, 0, 0]).astype(BF16NP)  # [1024,512] scaled bf16
    wsl = np.ascontiguousarray(ws_sc.T.reshape(4, 128, 1024))

    wb2 = np.zeros((128, 4608), np.float32)
    for tap in range(9):
        for k in range(2):
            for m in range(2):
                wb2[:, ((tap * 2 + k) * 2 + m) * 128 : ((tap * 2 + k) * 2 + m + 1) * 128] = (
                    w2l[tap, k][:, m * 128 : (m + 1) * 128]
                )
    wb3 = np.zeros((128, 6144), np.float32)
    for k in range(2):
        for m8 in range(8):
            wb3[:, (k * 8 + m8) * 128 : (k * 8 + m8 + 1) * 128] = w3l[k][
                :, m8 * 128 : (m8 + 1) * 128
            ]
    for k in range(4):
        for m8 in range(8):
            wb3[:, 2048 + (k * 8 + m8) * 128 : 2048 + (k * 8 + m8 + 1) * 128] = wsl[k][
                :, m8 * 128 : (m8 + 1) * 128
            ].astype(np.float32)

    # column sums for the +128 activation offset corrections (fp64 exact)
    K1 = w1q[:, :, 0, 0].astype(np.float64).sum(axis=1)  # [256]
    K2 = w2q.astype(np.float64).sum(axis=(1, 2, 3))  # [256]
    K3 = w3q[:, :, 0, 0].astype(np.float64).sum(axis=1)  # [1024]
    Ks = ws_sc.astype(np.float64).sum(axis=1)  # [1024] (bf16-rounded values)

    beta1 = (f32(4.0) * B1 - a1 * f32(128.0) * K1.astype(np.float32)).astype(
        np.float32
    )
    beta2 = (f32(4.0) * B2 - a2 * f32(128.0) * K2.astype(np.float32)).astype(
        np.float32
    )
    delta0 = B3 * c3s / (f32(127.0) * c2s) + Bs / f32(127.0)
    delta = (
        delta0 - g3 * (f32(128.0) * K3 + f32(128.0) * Ks).astype(np.float32)
    ).astype(np.float32)

    msc = np.zeros((128, 12), np.float32)
    msc[:, 0:2] = beta1.reshape(2, 128).T
    msc[:, 2:4] = beta2.reshape(2, 128).T
    msc[:, 4:12] = delta.reshape(8, 128).T

    return dict(
        wa=wa.astype(BF16NP), wb2=wb2.astype(BF16NP), wb3=wb3.astype(BF16NP),
        msc=msc, a1=float(a1), a2=float(a2), g3=float(g3),
    )


def _quant_x(x):
    """x [64,512,28,28] f32 -> codes+128 bf16 packed [8cores][4pair][4k][128][1568]."""
    codes = np.rint(np.clip(254.0 * x, -127.0, 127.0)).astype(np.float32) + 128.0
    # [64b, 512c, 784] -> [core(8), pair(4), i(2), k(4), q(128), hw(784)]
    c = codes.reshape(8, 4, 2, 4, 128, 784)
    # -> [core, pair, k, q, i, hw]
    c = c.transpose(0, 1, 3, 4, 2, 5)
    return np.ascontiguousarray(c.reshape(8, 4, 4, 128, 1568).astype(BF16NP))


def kernel(x, w1, b1, w2, b2, w3, b3, ws, bs):
    global _SCALES
    x = np.asarray(x, dtype=np.float32)
    pre = _prep(
        np.asarray(w1, np.float32), np.asarray(b1, np.float32),
        np.asarray(w2, np.float32), np.asarray(b2, np.float32),
        np.asarray(w3, np.float32), np.asarray(b3, np.float32),
        np.asarray(ws, np.float32), np.asarray(bs, np.float32),
    )
    _SCALES = (pre["a1"], pre["a2"], pre["g3"])
    nc = _build_nc()
    nc.compile()

    xall = _quant_x(x)
    shared = {
        "wa": pre["wa"], "wb2": pre["wb2"], "wb3": pre["wb3"], "msc": pre["msc"],
    }
    in_maps = [{"xc": xall[c], **shared} for c in range(N_CORES)]

    tmpdir = os.environ.get("KERNEL_TRACE_DIR") or None
    if tmpdir:
        os.makedirs(tmpdir, exist_ok=True)
    res = run_bass_kernel_spmd(nc, in_maps, list(range(N_CORES)), tmpdir=tmpdir)
    global LAST_RESULT
    LAST_RESULT = res
    outs = [unpack_y(res.results[c]["y"]) for c in range(N_CORES)]
    return np.ascontiguousarray(np.concatenate(outs, axis=0))


def unpack_y(y):
    """[pairs,128,3136] f16 -> [8, 1024, 14, 14] f32."""
    p = y.shape[0]
    y = np.asarray(y).reshape(p, 128, 8, 2, 196).astype(np.float32)
    y = y.transpose(0, 3, 2, 1, 4)  # (pair, i, m8, q, hw)
    return np.ascontiguousarray(y.reshape(2 * p, 1024, 14, 14))
